# revision 54
# baseline (speedup 1.0000x reference)
"""Bass/Trainium2 kernel for a 6-layer GPT-style transformer (BigramLanguageModel).

Contract: kernel(**inputs) takes the FULL unsharded inputs from
reference.setup_inputs() and returns the FULL [32, 512, 65] fp32 logits.

Sharding: data-parallel over batch. Each of the 8 NeuronCores runs the whole
model on 4 of the 32 sequences (params replicated); outputs are concatenated
on the host. No collectives.

Device-side design (per core, 2048 tokens), v2 (all-bf16 matmuls):
 - all matmul operands bf16 (weights converted on host; activations written
   bf16 at the PSUM->SBUF copy). PSUM accumulation stays fp32. This enables
   FWL weight loads, 1 cyc/row matmuls everywhere, and 2x/4x DVE modes.
 - residual stream x: fp32 token-major SBUF [128, 16, 384].
 - LayerNorm: bn_stats/bn_aggr (DVE) in groups of 8 token tiles; apply writes
   bf16 h; PE transposes 8 tiles/chunk into one 2-bank PSUM tile; single
   [128,1024] DVE copy to the E-major hT buffer.
 - QKV: QT/KT per head-pair [128, 2048] bf16, head0 on partitions 0:64 and
   head1 on 64:128 (one PSUM->SBUF copy per 2 n-blocks); V token-major into
   v_aug [128, 16, 768] with per-head windows [ones64 | V_h64].
 - attention per (pair, seq): both heads' S^T blocks [128 k, width] computed
   into the two banks of one PSUM tile by row-packed concurrent K=64 matmuls;
   causal mask applied by PRE-ACCUMULATING -1e30 into the diagonal 128 cols
   via an ident@negtri matmul (start=True) so exp(scale*S) lands 0 exactly --
   no DVE mask op, chain is S(PE)->exp(ACT)->AV(PE). Batched exp over both
   heads [128, 2, width]. AV uses the [ones|V] stationary windows so PSUM
   rows 0:64 replicate the softmax denominator -> one batched fast-reciprocal
   + per-head tensor_tensor writes the normalized OT into the E-major concat
   buffer feeding the proj matmul.
 - MLP: mlpT = W1.T @ h2T (E-major), relu fused into the PSUM->SBUF copy
   (bf16), W2 with mlpT chunks stationary, token-major out + residual add.
 - logits: final LN -> xfT -> x @ Wlm per token tile, DMA out [2048, 65].
 - single PSUM pool of [128, 2, 512] (2-bank) tiles, bufs=4 = all 8 banks.
"""

import sys

for _p in ("/opt/trn_rl_repo", "/opt/pypackages"):
    if _p not in sys.path:
        sys.path.insert(0, _p)

import ml_dtypes
import numpy as np

import concourse.bass as bass
import concourse.tile as tile
from concourse import bacc, mybir
from concourse.bass_utils import run_bass_kernel_spmd

F32 = mybir.dt.float32
BF16 = mybir.dt.bfloat16

N_EMBED = 384
CONTEXT = 512
N_HEADS = 6
HEAD_DIM = 64
N_LAYERS = 6
VOCAB = 65
B, T = 32, 512
LN_EPS = 1e-5
N_CORES = 8
B_LOC = B // N_CORES          # 4 sequences per core
N_TOK = B_LOC * T             # 2048 tokens per core
N_TILES = N_TOK // 128        # 16 token tiles
N_CHUNKS = N_EMBED // 128     # 3 E-chunks
N_MLP = 4 * N_EMBED           # 1536
N_MCHUNK = N_MLP // 128       # 12
SCALE = float(N_EMBED) ** -0.5
# Mask addend: scale*NEG ~ -102 -> exp underflows to 0 (exact 0 after bf16
# cast). Huge magnitudes (-1e30) make the HW ACT exp LUT produce NaN.
NEG = -2000.0
DEBUG_L0 = False
V_W = N_HEADS * 128           # 768: per-head [ones64 | V64] windows


def _prep(inputs):
    """Host-side layout prep + exact LN folds. Returns (shared, has, idx)."""
    f = lambda a: np.ascontiguousarray(np.asarray(a), dtype=np.float32)
    bf = lambda a: np.ascontiguousarray(np.asarray(a)).astype(np.float32)
    idx = np.asarray(inputs["idx"])
    tok_emb, pos_emb = f(inputs["tok_emb"]), f(inputs["pos_emb"])
    Wq, Wk, Wv = f(inputs["Wq"]), f(inputs["Wk"]), f(inputs["Wv"])
    Wproj, bproj = f(inputs["Wproj"]), f(inputs["bproj"])
    W1, b1, W2, b2 = f(inputs["W1"]), f(inputs["b1"]), f(inputs["W2"]), f(inputs["b2"])
    ln1_g, ln1_b = f(inputs["ln1_g"]), f(inputs["ln1_b"])
    ln2_g, ln2_b = f(inputs["ln2_g"]), f(inputs["ln2_b"])
    lnf_g, lnf_b = f(inputs["lnf_g"]), f(inputs["lnf_b"])
    Wlm, blm = f(inputs["Wlm"]), f(inputs["blm"])

    L, H, E, D = N_LAYERS, N_HEADS, N_EMBED, HEAD_DIM

    # fold ln gains into the consuming weights (exact when g==1)
    Wq_f = ln1_g[:, None, :, None] * Wq          # [L,H,E,D]
    Wk_f = ln1_g[:, None, :, None] * Wk
    Wv_f = ln1_g[:, None, :, None] * Wv
    W1_f = ln2_g[:, :, None] * W1                # [L,E,4E]
    Wlm_f = lnf_g[:, None] * Wlm                 # [E,V]

    # ln biases propagate through the matmuls as constant bias vectors
    qb = np.einsum("le,lhed->lhd", ln1_b, Wq)    # [L,H,D]
    kb = np.einsum("le,lhed->lhd", ln1_b, Wk)
    vb = np.einsum("le,lhed->lhd", ln1_b, Wv)
    b1_eff = b1 + np.einsum("le,lem->lm", ln2_b, W1)    # [L,4E]
    blm_eff = blm + lnf_b @ Wlm                          # [V]

    # head-pair packed QT/KT weights: [L, 3, E, 128] (pair r = heads 2r, 2r+1)
    wqp = np.concatenate([Wq_f[:, 0::2], Wq_f[:, 1::2]], axis=-1)
    wkp = np.concatenate([Wk_f[:, 0::2], Wk_f[:, 1::2]], axis=-1)
    qbp = np.concatenate([qb[:, 0::2], qb[:, 1::2]], axis=-1)      # [L,3,128]
    kbp = np.concatenate([kb[:, 0::2], kb[:, 1::2]], axis=-1)
    wv_all = Wv_f.transpose(0, 2, 1, 3).reshape(L, E, H * D)       # [L,E,384]
    vb_all = vb.reshape(L, H * D)

    # negtri[k, q] = -1e30 where key k > query q (strict upper kept at 0)
    triu = np.triu(np.ones((128, 128), dtype=np.float32))
    negtri = (1.0 - triu) * NEG

    b16 = lambda a: np.ascontiguousarray(a).astype(ml_dtypes.bfloat16)
    shared = dict(
        tok_emb=b16(tok_emb),
        pos_emb=b16(pos_emb),
        wqp=b16(wqp),
        wkp=b16(wkp),
        wv=b16(wv_all),
        wp=b16(Wproj),
        w1=b16(W1_f),
        w2=b16(W2),
        wlm=b16(Wlm_f),
        ident=b16(np.eye(128, dtype=np.float32)),
        iota=np.arange(VOCAB, dtype=np.float32).reshape(VOCAB, 1),
        negtri=b16(negtri),
    )
    flags = dict(
        qb=qbp if np.any(qbp) else None,
        kb=kbp if np.any(kbp) else None,
        vb=np.broadcast_to(vb_all[:, None, :], (L, 128, H * D)).copy()
        if np.any(vb) else None,
        bp=np.broadcast_to(bproj[:, None, :], (L, 128, E)).copy()
        if np.any(bproj) else None,
        b1=np.ascontiguousarray(b1_eff.reshape(L, N_MCHUNK, 128).transpose(0, 2, 1))
        if np.any(b1_eff) else None,                    # [L,128,12] partition-major
        b2=np.broadcast_to(b2[:, None, :], (L, 128, E)).copy() if np.any(b2) else None,
        blm=np.broadcast_to(blm_eff[None, :], (128, VOCAB)).copy()
        if np.any(blm_eff) else None,
    )
    for k, v in flags.items():
        if v is not None:
            shared[k] = np.ascontiguousarray(v, dtype=np.float32)
    has = {k: (v is not None) for k, v in flags.items()}

    idx_f = idx.astype(np.float32).reshape(N_CORES, N_TOK)
    return shared, has, idx_f


def _build(has):
    nc = bacc.Bacc(trn_type="TRN2", debug=False, num_devices=N_CORES)
    d = {}
    d["idxf"] = nc.dram_tensor("idxf", [N_TOK], F32, kind="ExternalInput")
    d["tok_emb"] = nc.dram_tensor("tok_emb", [VOCAB, N_EMBED], BF16, kind="ExternalInput")
    d["pos_emb"] = nc.dram_tensor("pos_emb", [CONTEXT, N_EMBED], BF16, kind="ExternalInput")
    d["wqp"] = nc.dram_tensor("wqp", [N_LAYERS, 3, N_EMBED, 128], BF16, kind="ExternalInput")
    d["wkp"] = nc.dram_tensor("wkp", [N_LAYERS, 3, N_EMBED, 128], BF16, kind="ExternalInput")
    d["wv"] = nc.dram_tensor("wv", [N_LAYERS, N_EMBED, N_EMBED], BF16, kind="ExternalInput")
    d["wp"] = nc.dram_tensor("wp", [N_LAYERS, N_EMBED, N_EMBED], BF16, kind="ExternalInput")
    d["w1"] = nc.dram_tensor("w1", [N_LAYERS, N_EMBED, N_MLP], BF16, kind="ExternalInput")
    d["w2"] = nc.dram_tensor("w2", [N_LAYERS, N_MLP, N_EMBED], BF16, kind="ExternalInput")
    d["wlm"] = nc.dram_tensor("wlm", [N_EMBED, VOCAB], BF16, kind="ExternalInput")
    d["ident"] = nc.dram_tensor("ident", [128, 128], BF16, kind="ExternalInput")
    d["iota"] = nc.dram_tensor("iota", [VOCAB, 1], F32, kind="ExternalInput")
    d["negtri"] = nc.dram_tensor("negtri", [128, 128], BF16, kind="ExternalInput")
    if has["qb"]:
        d["qb"] = nc.dram_tensor("qb", [N_LAYERS, 3, 128], F32, kind="ExternalInput")
    if has["kb"]:
        d["kb"] = nc.dram_tensor("kb", [N_LAYERS, 3, 128], F32, kind="ExternalInput")
    if has["vb"]:
        d["vb"] = nc.dram_tensor("vb", [N_LAYERS, 128, N_EMBED], F32, kind="ExternalInput")
    if has["bp"]:
        d["bp"] = nc.dram_tensor("bp", [N_LAYERS, 128, N_EMBED], F32, kind="ExternalInput")
    if has["b1"]:
        d["b1"] = nc.dram_tensor("b1", [N_LAYERS, 128, N_MCHUNK], F32, kind="ExternalInput")
    if has["b2"]:
        d["b2"] = nc.dram_tensor("b2", [N_LAYERS, 128, N_EMBED], F32, kind="ExternalInput")
    if has["blm"]:
        d["blm"] = nc.dram_tensor("blm", [128, VOCAB], F32, kind="ExternalInput")
    logits_d = nc.dram_tensor("logits", [N_TOK, VOCAB], F32, kind="ExternalOutput")
    dbg = {}
    if DEBUG_L0:
        dbg["x0"] = nc.dram_tensor("dbg_x0", [128, N_TILES * N_EMBED], BF16, kind="ExternalOutput")
        dbg["ht"] = nc.dram_tensor("dbg_ht", [128, N_CHUNKS * N_TOK], BF16, kind="ExternalOutput")
        dbg["qt"] = nc.dram_tensor("dbg_qt", [128, N_TOK], BF16, kind="ExternalOutput")
        dbg["kt"] = nc.dram_tensor("dbg_kt", [128, N_TOK], BF16, kind="ExternalOutput")
        dbg["va"] = nc.dram_tensor("dbg_va", [128, N_TILES * V_W], BF16, kind="ExternalOutput")
        dbg["at0"] = nc.dram_tensor("dbg_at0", [128, 2 * 512], BF16, kind="ExternalOutput")
        dbg["po0"] = nc.dram_tensor("dbg_po0", [128, 2 * 512], F32, kind="ExternalOutput")
        dbg["rho0"] = nc.dram_tensor("dbg_rho0", [64, 2 * 512], F32, kind="ExternalOutput")
        dbg["otc"] = nc.dram_tensor("dbg_otc", [128, N_CHUNKS * N_TOK], BF16, kind="ExternalOutput")
        dbg["x1"] = nc.dram_tensor("dbg_x1", [128, N_TILES * N_EMBED], BF16, kind="ExternalOutput")

    AF = mybir.ActivationFunctionType
    OP = mybir.AluOpType

    with tile.TileContext(nc) as tc:
        with tc.tile_pool(name="const", bufs=1) as cst, \
             tc.tile_pool(name="persist", bufs=1) as per, \
             tc.tile_pool(name="work", bufs=3) as wk, \
             tc.tile_pool(name="htile", bufs=9) as hp, \
             tc.tile_pool(name="wts", bufs=4) as wts, \
             tc.tile_pool(name="ps", bufs=4, space="PSUM") as ps:

            # ---- constants ----
            ident = cst.tile([128, 128], BF16)
            nc.sync.dma_start(ident, d["ident"][:, :])
            iota = cst.tile([VOCAB, 1], F32)
            nc.sync.dma_start(iota, d["iota"][:, :])
            negtri = cst.tile([128, 128], BF16)
            nc.sync.dma_start(negtri, d["negtri"][:, :])
            eps_sb = cst.tile([128, 1], F32)
            nc.vector.memset(eps_sb, LN_EPS)
            tok_sb = cst.tile([VOCAB, N_EMBED], BF16)
            nc.sync.dma_start(tok_sb, d["tok_emb"][:, :])

            bias_sb = {}
            for nm, shp in (("vb", [128, N_EMBED]), ("bp", [128, N_EMBED]),
                            ("b2", [128, N_EMBED])):
                if has[nm]:
                    bias_sb[nm] = cst.tile([128, N_LAYERS, shp[1]], F32)
                    nc.sync.dma_start(
                        bias_sb[nm],
                        d[nm].rearrange("l p e -> p l e"))
            if has["b1"]:
                bias_sb["b1"] = cst.tile([128, N_LAYERS, N_MCHUNK], F32)
                nc.sync.dma_start(bias_sb["b1"], d["b1"].rearrange("l p m -> p l m"))
            for nm in ("qb", "kb"):
                if has[nm]:
                    bias_sb[nm] = cst.tile([128, N_LAYERS, 3], F32)
                    nc.sync.dma_start(bias_sb[nm], d[nm].rearrange("l r p -> p l r"))
            if has["blm"]:
                bias_sb["blm"] = cst.tile([128, VOCAB], F32)
                nc.sync.dma_start(bias_sb["blm"], d["blm"][:, :])

            # ---- persistent activations ----
            x = per.tile([128, N_TILES, N_EMBED], BF16)         # residual, token-major
            pos_sb = cst.tile([128, B_LOC, N_EMBED], BF16)
            nc.sync.dma_start(
                pos_sb, d["pos_emb"].rearrange("(a p) e -> p a e", p=128))
            v_aug = per.tile([128, N_TILES, V_W], BF16)         # per-head [ones|V]
            ones_blk = cst.tile([128, 64], BF16)
            nc.vector.memset(ones_blk, 1.0)
            for h in range(N_HEADS):                            # ones stripes
                nc.vector.tensor_copy(
                    v_aug[:, :, h * 128:h * 128 + 64],
                    ones_blk[:, None, :].to_broadcast([128, N_TILES, 64]))

            # ---- embedding: x = onehot(idx) @ tok_emb + pos ----
            for tp in range(N_TILES // 2):
                pe = ps.tile([128, 2, 512], F32, tag="ps")
                for dt in range(2):
                    t = tp * 2 + dt
                    idx_b = wk.tile([VOCAB, 128], F32, tag="idxb")
                    nc.sync.dma_start(
                        idx_b,
                        bass.AP(tensor=d["idxf"], offset=t * 128,
                                ap=[[0, VOCAB], [1, 128]]))
                    oh = wk.tile([VOCAB, 128], BF16, tag="oh")
                    nc.vector.tensor_scalar(out=oh, in0=idx_b, scalar1=iota,
                                            scalar2=None, op0=OP.is_equal)
                    nc.tensor.matmul(pe[:, dt, :N_EMBED], lhsT=oh, rhs=tok_sb,
                                     start=True, stop=False)
                    nc.tensor.matmul(pe[:, dt, :N_EMBED], lhsT=ident,
                                     rhs=pos_sb[:, t % B_LOC, :],
                                     start=False, stop=True)
                    nc.scalar.copy(x[:, t, :], pe[:, dt, :N_EMBED])

            LNG = 4

            def ln_group(dst_hT, src_name, tg):
                """One LN group: stats -> rstd -> apply (bf16) -> 12 PE
                transposes into one 2-bank bf16 PSUM tile (chunks 0,1 in
                bank0, chunk 2 in bank1) -> 2 DVE copies to E-major dst."""
                G = LNG
                mvg = wk.tile([128, G, 2], F32, tag="mv" + src_name)
                for dt in range(G):
                    st = wk.tile([128, 6], F32, tag="bnst")
                    nc.vector.bn_stats(out=st, in_=x[:, tg * G + dt, :])
                    nc.vector.bn_aggr(out=mvg[:, dt, :], in_=st)
                sstd = wk.tile([128, G], F32, tag="sstd")
                nc.scalar.activation(out=sstd, in_=mvg[:, :, 1],
                                     func=AF.Sqrt, bias=eps_sb, scale=1.0)
                rstd = wk.tile([128, G], F32, tag="rstd")
                nc.vector.reciprocal(out=rstd, in_=sstd)
                hts = []
                for dt in range(G):
                    t = tg * G + dt
                    ht = hp.tile([128, N_EMBED], BF16, tag="h")
                    nc.vector.tensor_scalar(
                        out=ht, in0=x[:, t, :],
                        scalar1=mvg[:, dt, 0:1], scalar2=rstd[:, dt:dt + 1],
                        op0=OP.subtract, op1=OP.mult)
                    hts.append(ht)
                pt = ps.tile([128, 2, 1024], BF16, tag="ps")
                for c in range(N_CHUNKS):
                    for dt in range(G):
                        nc.tensor.transpose(
                            pt[:, c // 2, (c % 2) * 512 + dt * 128:
                               (c % 2) * 512 + (dt + 1) * 128],
                            hts[dt][:, c * 128:(c + 1) * 128], ident)
                nc.vector.tensor_copy(
                    dst_hT[:, 0:2, tg * G * 128:(tg + 1) * G * 128],
                    pt[:, 0, :].rearrange("p (a b) -> p a b", a=2))
                nc.vector.tensor_copy(
                    dst_hT[:, 2, tg * G * 128:(tg + 1) * G * 128],
                    pt[:, 1, 0:512])

            def layernorm_to(dst_hT, src_name):
                for tg in range(N_TILES // LNG):
                    ln_group(dst_hT, src_name, tg)

            for layer in range(N_LAYERS):
                if DEBUG_L0 and layer == 0:
                    nc.sync.dma_start(dbg["x0"][:, :], x.rearrange("p a b -> p (a b)"))
                hT = per.tile([128, N_CHUNKS, N_TOK], BF16, tag="ht1")

                # ---- V weights ----
                wv_c = []
                for c in range(N_CHUNKS):
                    w = wts.tile([128, N_EMBED], BF16, tag="wchk", bufs=9)
                    nc.sync.dma_start(w, d["wv"][layer, c * 128:(c + 1) * 128, :])
                    wv_c.append(w)

                def v_pair(tp):
                    pv = ps.tile([128, 2, 512], F32, tag="ps")
                    for dt in range(2):
                        for c in range(N_CHUNKS):
                            nc.tensor.matmul(pv[:, dt, :N_EMBED],
                                             lhsT=hT[:, c, (tp * 2 + dt) * 128:
                                                     (tp * 2 + dt + 1) * 128],
                                             rhs=wv_c[c],
                                             start=(c == 0), stop=(c == N_CHUNKS - 1))
                    # scatter [128, 2, 6, 64] -> per-head V slots (offset 64)
                    src = pv[:, :, :N_EMBED].rearrange("p a (h j) -> p a h j", h=6)
                    dst = v_aug[:, tp * 2:tp * 2 + 2, :].rearrange(
                        "p a (h j) -> p a h j", h=6)[:, :, :, 64:128]
                    if has["vb"]:
                        nc.vector.tensor_tensor(
                            out=dst, in0=src,
                            in1=bias_sb["vb"][:, layer, :].rearrange(
                                "p (h j) -> p h j", h=6)[:, None, :, :]
                            .to_broadcast([128, 2, 6, 64]),
                            op=OP.add)
                    else:
                        nc.scalar.copy(dst, src)

                otc = per.tile([128, N_CHUNKS, N_TOK], BF16, tag="otc")

                def emit_qk_chunks(pair):
                    # ---- QT/KT for this pair: [128, 2048] bf16, head0 on
                    # partitions 0:64, head1 on 64:128. Returns (qkt, list of
                    # 4 chunk closures) so chunks can interleave with the
                    # previous pair's attention groups. ----
                    qkt, chunks = {}, []
                    for nm, wd, bias_nm in (("q", d["wqp"], "qb"),
                                            ("k", d["wkp"], "kb")):
                        wqk = wts.tile([128, N_CHUNKS, 128], BF16, tag="wqk",
                                       bufs=4, name=f"wqk_{nm}")
                        for c in range(N_CHUNKS):
                            nc.sync.dma_start(
                                wqk[:, c, :],
                                wd[layer, pair, c * 128:(c + 1) * 128, :])
                        dstT = per.tile([128, N_TOK], BF16, tag=nm + "t",
                                        bufs=3, name=f"qk_{nm}")
                        qkt[nm] = dstT

                        def chunk(np_, wqk=wqk, dstT=dstT, bias_nm=bias_nm):
                            pq = ps.tile([128, 2, 512], F32, tag="ps", name="pq")
                            for half in range(2):
                                n = np_ * 2 + half
                                for c in range(N_CHUNKS):
                                    nc.tensor.matmul(
                                        pq[:, half, :],
                                        lhsT=wqk[:, c, :],
                                        rhs=hT[:, c, n * 512:(n + 1) * 512],
                                        start=(c == 0), stop=(c == N_CHUNKS - 1))
                            for half in range(2):
                                n = np_ * 2 + half
                                dst = dstT[:, n * 512:(n + 1) * 512]
                                if has[bias_nm]:
                                    nc.scalar.activation(
                                        out=dst, in_=pq[:, half, :],
                                        func=AF.Identity,
                                        bias=bias_sb[bias_nm][:, layer,
                                                             pair:pair + 1],
                                        scale=1.0)
                                else:
                                    nc.scalar.copy(dst, pq[:, half, :])

                        for np_ in range(N_TOK // 1024):
                            chunks.append(lambda np_=np_, chunk=chunk: chunk(np_))
                    return qkt, chunks

                def att_pair(pair, qkt, s0, s1):
                    # ---- attention for two seqs of one head-pair, ki-level
                    # interleaved so the PE stays dense while ACT runs exps.
                    # 3 steps: ki=0 (512), ki=1 (384), ki={2,3} packed into
                    # one bank (cols 0:256 + 256:384) -> 3 exps per group.
                    # S^T for head j lands in bank j of pa; causal mask is
                    # accumulated as -2000 onto each block's diagonal cols. ----
                    seqs = (s0, s1)
                    kT, qT = qkt["k"], qkt["q"]
                    pos = [ps.tile([128, 2, 512], F32, tag="ps", name="po")
                           for _ in range(2)]
                    for step in range(3):
                        pas, ats = {}, {}
                        for g in range(2):
                            s = seqs[g]
                            pa = ps.tile([128, 2, 512], F32, tag="ps", name="pa")
                            at2 = wk.tile([128, 2, 512], BF16, tag="at_sb",
                                          bufs=6, name="at")
                            pas[g], ats[g] = pa, at2
                            if step < 2:
                                ki = step
                                width = 512 - ki * 128
                                kc = s * 512 + ki * 128
                                for j in range(2):
                                    nc.tensor.matmul(
                                        pa[:, j, :width],
                                        lhsT=kT[j * 64:j * 64 + 64, kc:kc + 128],
                                        rhs=qT[j * 64:j * 64 + 64,
                                               kc:s * 512 + 512],
                                        start=True, stop=False)
                                for j in range(2):
                                    nc.tensor.matmul(
                                        pa[:, j, 0:128], lhsT=ident, rhs=negtri,
                                        start=False, stop=True)
                                    # per-head exp: head j's AV can start as
                                    # soon as its own mask lands
                                    nc.scalar.activation(
                                        out=at2[:, j, :width],
                                        in_=pa[:, j, :width],
                                        func=AF.Exp, scale=SCALE)
                            else:
                                kc3 = s * 512 + 384
                                kc2 = s * 512 + 256
                                for j in range(2):
                                    nc.tensor.matmul(
                                        pa[:, j, 256:384],
                                        lhsT=kT[j * 64:j * 64 + 64, kc3:kc3 + 128],
                                        rhs=qT[j * 64:j * 64 + 64,
                                               kc3:s * 512 + 512],
                                        start=True, stop=False)
                                for j in range(2):
                                    nc.tensor.matmul(
                                        pa[:, j, 256:384], lhsT=ident,
                                        rhs=negtri, start=False, stop=False)
                                # second start=True re-clears only the
                                # has_written bits; ki3 values persist
                                for j in range(2):
                                    nc.tensor.matmul(
                                        pa[:, j, 0:256],
                                        lhsT=kT[j * 64:j * 64 + 64, kc2:kc2 + 128],
                                        rhs=qT[j * 64:j * 64 + 64,
                                               kc2:s * 512 + 512],
                                        start=True, stop=False,
                                        skip_group_check=True)
                                for j in range(2):
                                    nc.tensor.matmul(
                                        pa[:, j, 0:128], lhsT=ident, rhs=negtri,
                                        start=False, stop=True)
                                    nc.scalar.activation(
                                        out=at2[:, j, 0:384],
                                        in_=pa[:, j, 0:384],
                                        func=AF.Exp, scale=SCALE)
                        for g in range(2):
                            s = seqs[g]
                            pa, at2 = pas[g], ats[g]
                            for j in range(2):
                                h = 2 * pair + j
                                if step < 2:
                                    ki = step
                                    nc.tensor.matmul(
                                        pos[g][:, j, ki * 128:512],
                                        lhsT=v_aug[:, s * 4 + ki,
                                                   h * 128:h * 128 + 128],
                                        rhs=at2[:, j, :512 - ki * 128],
                                        start=(ki == 0), stop=False)
                                else:
                                    nc.tensor.matmul(
                                        pos[g][:, j, 256:512],
                                        lhsT=v_aug[:, s * 4 + 2,
                                                   h * 128:h * 128 + 128],
                                        rhs=at2[:, j, 0:256],
                                        start=False, stop=False)
                                    nc.tensor.matmul(
                                        pos[g][:, j, 384:512],
                                        lhsT=v_aug[:, s * 4 + 3,
                                                   h * 128:h * 128 + 128],
                                        rhs=at2[:, j, 256:384],
                                        start=False, stop=True)
                    for g in range(2):
                        s = seqs[g]
                        po = pos[g]
                        # rows 0:64 of each bank replicate the denominator
                        rho = wk.tile([64, 2, 512], F32, tag="rho", bufs=2,
                                      name="rho")
                        nc.vector.reciprocal_approx_fast(
                            out=rho, in_=po[0:64, :, :])
                        for j in range(2):
                            nc.vector.tensor_tensor(
                                out=otc[64 * j:64 * j + 64, pair,
                                        s * 512:(s + 1) * 512],
                                in0=po[64:128, j, :], in1=rho[:, j, :],
                                op=OP.mult)

                # ---- proj weights (prefetch before attention tail) ----
                wp_c = []
                for c in range(N_CHUNKS):
                    w = wts.tile([128, N_EMBED], BF16, tag="wchk", bufs=9)
                    nc.sync.dma_start(w, d["wp"][layer, c * 128:(c + 1) * 128, :])
                    wp_c.append(w)

                def proj_pair(tp):
                    pp = ps.tile([128, 2, 512], F32, tag="ps")
                    for dt in range(2):
                        t = tp * 2 + dt
                        for c in range(N_CHUNKS):
                            nc.tensor.matmul(
                                pp[:, dt, :N_EMBED],
                                lhsT=otc[:, c, t * 128:(t + 1) * 128],
                                rhs=wp_c[c],
                                start=(c == 0), stop=False)
                        # residual: accumulate x into PSUM via identity matmul
                        nc.tensor.matmul(pp[:, dt, :N_EMBED], lhsT=ident,
                                         rhs=x[:, t, :], start=False, stop=True)
                    if has["bp"]:
                        nc.vector.tensor_tensor(
                            out=x[:, tp * 2:tp * 2 + 2, :],
                            in0=pp[:, :, :N_EMBED],
                            in1=bias_sb["bp"][:, None, layer, :]
                            .to_broadcast([128, 2, N_EMBED]), op=OP.add)
                    else:
                        nc.scalar.copy(x[:, tp * 2:tp * 2 + 2, :],
                                       pp[:, :, :N_EMBED])

                # LN1 groups interleave with V pairs and QK0 chunks; QK of
                # pair p+1 interleaves with attention of pair p (dense PE
                # work hides the exp latency); proj of seq s interleaves
                # with the last pair's attention.
                qkt0, ch0 = emit_qk_chunks(0)
                for tg in range(N_TILES // LNG):
                    ln_group(hT, "ln1", tg)
                    v_pair(2 * tg)
                    v_pair(2 * tg + 1)
                    if tg == 1:
                        ch0[0]()
                        ch0[2]()
                    elif tg == 3:
                        ch0[1]()
                        ch0[3]()
                if DEBUG_L0 and layer == 0:
                    nc.sync.dma_start(dbg["ht"][:, :], hT.rearrange("p a b -> p (a b)"))
                    nc.sync.dma_start(dbg["va"][:, :], v_aug.rearrange("p a b -> p (a b)"))
                # QK of pair p+1 interleaves with attention of pair p (dense
                # PE work hides the exp latency); proj of seq s interleaves
                # with the last pair's attention.
                qkt1, ch1 = emit_qk_chunks(1)
                att_pair(0, qkt0, 0, 1)
                ch1[0]()
                ch1[1]()
                att_pair(0, qkt0, 2, 3)
                ch1[2]()
                ch1[3]()
                qkt2, ch2 = emit_qk_chunks(2)
                att_pair(1, qkt1, 0, 1)
                ch2[0]()
                ch2[1]()
                att_pair(1, qkt1, 2, 3)
                ch2[2]()
                ch2[3]()
                att_pair(2, qkt2, 0, 1)
                for tp in range(4):
                    proj_pair(tp)
                att_pair(2, qkt2, 2, 3)
                for tp in range(4, 8):
                    proj_pair(tp)

                if DEBUG_L0 and layer == 0:
                    nc.sync.dma_start(dbg["otc"][:, :], otc.rearrange("p a b -> p (a b)"))
                    for nm_, t_ in (("qt", qkt2["q"]), ("kt", qkt2["k"])):
                        nc.sync.dma_start(dbg[nm_][:, :], t_[:, :])
                    nc.sync.dma_start(dbg["x1"][:, :], x.rearrange("p a b -> p (a b)"))

                # ---- MLP (LN2 group n feeds MLP1/MLP2 of n-block n) ----
                h2T = per.tile([128, N_CHUNKS, N_TOK], BF16, tag="ht2")
                w1all = wts.tile([128, N_CHUNKS, N_MLP], BF16, tag="w1all", bufs=2)
                for c in range(N_CHUNKS):
                    nc.sync.dma_start(
                        w1all[:, c, :], d["w1"][layer, c * 128:(c + 1) * 128, :])
                w2all = wts.tile([128, N_MCHUNK, N_EMBED], BF16, tag="w2all", bufs=2)
                for m in range(N_MCHUNK):
                    nc.sync.dma_start(
                        w2all[:, m, :], d["w2"][layer, m * 128:(m + 1) * 128, :])
                mlpT = per.tile([128, N_MCHUNK, 512], BF16, tag="mlpt")

                def mlp_block(n):
                    ln_group(h2T, "ln2", n)
                    for mp in range(N_MCHUNK // 2):
                        pm = ps.tile([128, 2, 512], F32, tag="ps")
                        for dm in range(2):
                            m = mp * 2 + dm
                            for c in range(N_CHUNKS):
                                nc.tensor.matmul(
                                    pm[:, dm, :],
                                    lhsT=w1all[:, c, m * 128:(m + 1) * 128],
                                    rhs=h2T[:, c, n * 512:(n + 1) * 512],
                                    start=(c == 0), stop=(c == N_CHUNKS - 1))
                        if has["b1"]:
                            for dm in range(2):
                                nc.scalar.activation(
                                    out=mlpT[:, mp * 2 + dm, :],
                                    in_=pm[:, dm, :], func=AF.Relu,
                                    bias=bias_sb["b1"][:, layer,
                                                       mp * 2 + dm:mp * 2 + dm + 1],
                                    scale=1.0)
                        else:
                            nc.scalar.activation(
                                out=mlpT[:, mp * 2:mp * 2 + 2, :], in_=pm,
                                func=AF.Relu, scale=1.0)
                    for dp in range(2):
                        pw = ps.tile([128, 2, 512], F32, tag="ps")
                        for dt in range(2):
                            t = n * 4 + dp * 2 + dt
                            for m in range(N_MCHUNK):
                                nc.tensor.matmul(
                                    pw[:, dt, :N_EMBED],
                                    lhsT=mlpT[:, m, (dp * 2 + dt) * 128:
                                              (dp * 2 + dt + 1) * 128],
                                    rhs=w2all[:, m, :],
                                    start=(m == 0), stop=False)
                            nc.tensor.matmul(pw[:, dt, :N_EMBED], lhsT=ident,
                                             rhs=x[:, t, :],
                                             start=False, stop=True)
                        t0 = n * 4 + dp * 2
                        if has["b2"]:
                            nc.vector.tensor_tensor(
                                out=x[:, t0:t0 + 2, :],
                                in0=pw[:, :, :N_EMBED],
                                in1=bias_sb["b2"][:, None, layer, :]
                                .to_broadcast([128, 2, N_EMBED]), op=OP.add)
                        else:
                            nc.scalar.copy(x[:, t0:t0 + 2, :],
                                           pw[:, :, :N_EMBED])

                for n in range(N_TOK // 512):
                    mlp_block(n)

            # ---- final LN + LM head ----
            xfT = per.tile([128, N_CHUNKS, N_TOK], BF16, tag="ht1")
            layernorm_to(xfT, "lnf")
            wlm_c = []
            for c in range(N_CHUNKS):
                w = wts.tile([128, VOCAB], BF16, tag="wlm", bufs=3)
                nc.sync.dma_start(w, d["wlm"][c * 128:(c + 1) * 128, :])
                wlm_c.append(w)
            for tp in range(N_TILES // 2):
                pl = ps.tile([128, 2, 512], F32, tag="ps")
                for dt in range(2):
                    for c in range(N_CHUNKS):
                        nc.tensor.matmul(
                            pl[:, dt, :VOCAB],
                            lhsT=xfT[:, c, (tp * 2 + dt) * 128:
                                     (tp * 2 + dt + 1) * 128],
                            rhs=wlm_c[c],
                            start=(c == 0), stop=(c == N_CHUNKS - 1))
                lg = wk.tile([128, 2, VOCAB], F32, tag="lg")
                if has["blm"]:
                    nc.vector.tensor_tensor(
                        out=lg, in0=pl[:, :, :VOCAB],
                        in1=bias_sb["blm"][:, None, :].to_broadcast(
                            [128, 2, VOCAB]), op=OP.add)
                else:
                    nc.vector.tensor_copy(lg, pl[:, :, :VOCAB])
                for dt in range(2):
                    t = tp * 2 + dt
                    nc.sync.dma_start(
                        logits_d[t * 128:(t + 1) * 128, :], lg[:, dt, :])

    nc.compile()
    return nc


_CACHE = {}


def _get_nc(has):
    key = tuple(sorted(has.items()))
    if key not in _CACHE:
        _CACHE[key] = _build(has)
    return _CACHE[key]


def kernel(**inputs):
    shared, has, idx_f = _prep(inputs)
    nc = _get_nc(has)
    in_maps = []
    for core in range(N_CORES):
        m = dict(shared)
        m["idxf"] = idx_f[core]
        in_maps.append(m)
    res = run_bass_kernel_spmd(nc, in_maps, core_ids=list(range(N_CORES)))
    out = np.stack([r["logits"].reshape(B_LOC, T, VOCAB) for r in res.results])
    return out.reshape(B, T, VOCAB)


# revision 55
# speedup vs baseline: 1.2442x; 1.2442x over previous
"""Bass/Trainium2 kernel for a 6-layer GPT-style transformer (BigramLanguageModel).

Contract: kernel(**inputs) takes the FULL unsharded inputs from
reference.setup_inputs() and returns the FULL [32, 512, 65] fp32 logits.

Sharding: data-parallel over batch. Each of the 8 NeuronCores runs the whole
model on 4 of the 32 sequences (params replicated); outputs are concatenated
on the host. No collectives.

Device-side design (per core, 2048 tokens), v2 (all-bf16 matmuls):
 - all matmul operands bf16 (weights converted on host; activations written
   bf16 at the PSUM->SBUF copy). PSUM accumulation stays fp32. This enables
   FWL weight loads, 1 cyc/row matmuls everywhere, and 2x/4x DVE modes.
 - residual stream x: fp32 token-major SBUF [128, 16, 384].
 - LayerNorm: bn_stats/bn_aggr (DVE) in groups of 8 token tiles; apply writes
   bf16 h; PE transposes 8 tiles/chunk into one 2-bank PSUM tile; single
   [128,1024] DVE copy to the E-major hT buffer.
 - QKV: QT/KT per head-pair [128, 2048] bf16, head0 on partitions 0:64 and
   head1 on 64:128 (one PSUM->SBUF copy per 2 n-blocks); V token-major into
   v_aug [128, 16, 768] with per-head windows [ones64 | V_h64].
 - attention per (pair, seq): both heads' S^T blocks [128 k, width] computed
   into the two banks of one PSUM tile by row-packed concurrent K=64 matmuls;
   causal mask applied by PRE-ACCUMULATING -1e30 into the diagonal 128 cols
   via an ident@negtri matmul (start=True) so exp(scale*S) lands 0 exactly --
   no DVE mask op, chain is S(PE)->exp(ACT)->AV(PE). Batched exp over both
   heads [128, 2, width]. AV uses the [ones|V] stationary windows so PSUM
   rows 0:64 replicate the softmax denominator -> one batched fast-reciprocal
   + per-head tensor_tensor writes the normalized OT into the E-major concat
   buffer feeding the proj matmul.
 - MLP: mlpT = W1.T @ h2T (E-major), relu fused into the PSUM->SBUF copy
   (bf16), W2 with mlpT chunks stationary, token-major out + residual add.
 - logits: final LN -> xfT -> x @ Wlm per token tile, DMA out [2048, 65].
 - single PSUM pool of [128, 2, 512] (2-bank) tiles, bufs=4 = all 8 banks.
"""

import sys

for _p in ("/opt/trn_rl_repo", "/opt/pypackages"):
    if _p not in sys.path:
        sys.path.insert(0, _p)

import ml_dtypes
import numpy as np

import concourse.bass as bass
import concourse.tile as tile
from concourse import bacc, mybir
from concourse.bass_utils import run_bass_kernel_spmd

F32 = mybir.dt.float32
BF16 = mybir.dt.bfloat16

N_EMBED = 384
CONTEXT = 512
N_HEADS = 6
HEAD_DIM = 64
N_LAYERS = 6
VOCAB = 65
B, T = 32, 512
LN_EPS = 1e-5
N_CORES = 8
B_LOC = B // N_CORES          # 4 sequences per core
N_TOK = B_LOC * T             # 2048 tokens per core
N_TILES = N_TOK // 128        # 16 token tiles
N_CHUNKS = N_EMBED // 128     # 3 E-chunks
N_MLP = 4 * N_EMBED           # 1536
N_MCHUNK = N_MLP // 128       # 12
SCALE = float(N_EMBED) ** -0.5
# Mask addend: scale*NEG ~ -102 -> exp underflows to 0 (exact 0 after bf16
# cast). Huge magnitudes (-1e30) make the HW ACT exp LUT produce NaN.
NEG = -2000.0
DEBUG_L0 = False
V_W = N_HEADS * 128           # 768: per-head [ones64 | V64] windows


def _prep(inputs):
    """Host-side layout prep + exact LN folds. Returns (shared, has, idx)."""
    f = lambda a: np.ascontiguousarray(np.asarray(a), dtype=np.float32)
    bf = lambda a: np.ascontiguousarray(np.asarray(a)).astype(np.float32)
    idx = np.asarray(inputs["idx"])
    tok_emb, pos_emb = f(inputs["tok_emb"]), f(inputs["pos_emb"])
    Wq, Wk, Wv = f(inputs["Wq"]), f(inputs["Wk"]), f(inputs["Wv"])
    Wproj, bproj = f(inputs["Wproj"]), f(inputs["bproj"])
    W1, b1, W2, b2 = f(inputs["W1"]), f(inputs["b1"]), f(inputs["W2"]), f(inputs["b2"])
    ln1_g, ln1_b = f(inputs["ln1_g"]), f(inputs["ln1_b"])
    ln2_g, ln2_b = f(inputs["ln2_g"]), f(inputs["ln2_b"])
    lnf_g, lnf_b = f(inputs["lnf_g"]), f(inputs["lnf_b"])
    Wlm, blm = f(inputs["Wlm"]), f(inputs["blm"])

    L, H, E, D = N_LAYERS, N_HEADS, N_EMBED, HEAD_DIM

    # fold ln gains into the consuming weights (exact when g==1)
    Wq_f = ln1_g[:, None, :, None] * Wq          # [L,H,E,D]
    Wk_f = ln1_g[:, None, :, None] * Wk
    Wv_f = ln1_g[:, None, :, None] * Wv
    W1_f = ln2_g[:, :, None] * W1                # [L,E,4E]
    Wlm_f = lnf_g[:, None] * Wlm                 # [E,V]

    # ln biases propagate through the matmuls as constant bias vectors
    qb = np.einsum("le,lhed->lhd", ln1_b, Wq)    # [L,H,D]
    kb = np.einsum("le,lhed->lhd", ln1_b, Wk)
    vb = np.einsum("le,lhed->lhd", ln1_b, Wv)
    b1_eff = b1 + np.einsum("le,lem->lm", ln2_b, W1)    # [L,4E]
    blm_eff = blm + lnf_b @ Wlm                          # [V]

    # head-pair packed QT/KT weights: [L, 3, E, 128] (pair r = heads 2r, 2r+1)
    wqp = np.concatenate([Wq_f[:, 0::2], Wq_f[:, 1::2]], axis=-1)
    wkp = np.concatenate([Wk_f[:, 0::2], Wk_f[:, 1::2]], axis=-1)
    qbp = np.concatenate([qb[:, 0::2], qb[:, 1::2]], axis=-1)      # [L,3,128]
    kbp = np.concatenate([kb[:, 0::2], kb[:, 1::2]], axis=-1)
    wv_all = Wv_f.transpose(0, 2, 1, 3).reshape(L, E, H * D)       # [L,E,384]
    vb_all = vb.reshape(L, H * D)

    # negtri[k, q] = -1e30 where key k > query q (strict upper kept at 0)
    triu = np.triu(np.ones((128, 128), dtype=np.float32))
    negtri = (1.0 - triu) * NEG

    b16 = lambda a: np.ascontiguousarray(a).astype(ml_dtypes.bfloat16)
    shared = dict(
        tok_emb=b16(tok_emb),
        pos_emb=b16(pos_emb),
        wqp=b16(wqp),
        wkp=b16(wkp),
        wv=b16(wv_all),
        wp=b16(Wproj),
        w1=b16(W1_f),
        w2=b16(W2),
        wlm=b16(Wlm_f),
        ident=b16(np.eye(128, dtype=np.float32)),
        iota=np.arange(VOCAB, dtype=np.float32).reshape(VOCAB, 1),
        negtri=b16(negtri),
    )
    flags = dict(
        qb=qbp if np.any(qbp) else None,
        kb=kbp if np.any(kbp) else None,
        vb=np.broadcast_to(vb_all[:, None, :], (L, 128, H * D)).copy()
        if np.any(vb) else None,
        bp=np.broadcast_to(bproj[:, None, :], (L, 128, E)).copy()
        if np.any(bproj) else None,
        b1=np.ascontiguousarray(b1_eff.reshape(L, N_MCHUNK, 128).transpose(0, 2, 1))
        if np.any(b1_eff) else None,                    # [L,128,12] partition-major
        b2=np.broadcast_to(b2[:, None, :], (L, 128, E)).copy() if np.any(b2) else None,
        blm=np.broadcast_to(blm_eff[None, :], (128, VOCAB)).copy()
        if np.any(blm_eff) else None,
    )
    for k, v in flags.items():
        if v is not None:
            shared[k] = np.ascontiguousarray(v, dtype=np.float32)
    has = {k: (v is not None) for k, v in flags.items()}

    idx_f = idx.astype(np.float32).reshape(N_CORES, N_TOK)
    return shared, has, idx_f


def _build(has):
    nc = bacc.Bacc(trn_type="TRN2", debug=False, num_devices=N_CORES)
    d = {}
    d["idxf"] = nc.dram_tensor("idxf", [N_TOK], F32, kind="ExternalInput")
    d["tok_emb"] = nc.dram_tensor("tok_emb", [VOCAB, N_EMBED], BF16, kind="ExternalInput")
    d["pos_emb"] = nc.dram_tensor("pos_emb", [CONTEXT, N_EMBED], BF16, kind="ExternalInput")
    d["wqp"] = nc.dram_tensor("wqp", [N_LAYERS, 3, N_EMBED, 128], BF16, kind="ExternalInput")
    d["wkp"] = nc.dram_tensor("wkp", [N_LAYERS, 3, N_EMBED, 128], BF16, kind="ExternalInput")
    d["wv"] = nc.dram_tensor("wv", [N_LAYERS, N_EMBED, N_EMBED], BF16, kind="ExternalInput")
    d["wp"] = nc.dram_tensor("wp", [N_LAYERS, N_EMBED, N_EMBED], BF16, kind="ExternalInput")
    d["w1"] = nc.dram_tensor("w1", [N_LAYERS, N_EMBED, N_MLP], BF16, kind="ExternalInput")
    d["w2"] = nc.dram_tensor("w2", [N_LAYERS, N_MLP, N_EMBED], BF16, kind="ExternalInput")
    d["wlm"] = nc.dram_tensor("wlm", [N_EMBED, VOCAB], BF16, kind="ExternalInput")
    d["ident"] = nc.dram_tensor("ident", [128, 128], BF16, kind="ExternalInput")
    d["iota"] = nc.dram_tensor("iota", [VOCAB, 1], F32, kind="ExternalInput")
    d["negtri"] = nc.dram_tensor("negtri", [128, 128], BF16, kind="ExternalInput")
    if has["qb"]:
        d["qb"] = nc.dram_tensor("qb", [N_LAYERS, 3, 128], F32, kind="ExternalInput")
    if has["kb"]:
        d["kb"] = nc.dram_tensor("kb", [N_LAYERS, 3, 128], F32, kind="ExternalInput")
    if has["vb"]:
        d["vb"] = nc.dram_tensor("vb", [N_LAYERS, 128, N_EMBED], F32, kind="ExternalInput")
    if has["bp"]:
        d["bp"] = nc.dram_tensor("bp", [N_LAYERS, 128, N_EMBED], F32, kind="ExternalInput")
    if has["b1"]:
        d["b1"] = nc.dram_tensor("b1", [N_LAYERS, 128, N_MCHUNK], F32, kind="ExternalInput")
    if has["b2"]:
        d["b2"] = nc.dram_tensor("b2", [N_LAYERS, 128, N_EMBED], F32, kind="ExternalInput")
    if has["blm"]:
        d["blm"] = nc.dram_tensor("blm", [128, VOCAB], F32, kind="ExternalInput")
    logits_d = nc.dram_tensor("logits", [N_TOK, VOCAB], F32, kind="ExternalOutput")
    dbg = {}
    if DEBUG_L0:
        dbg["x0"] = nc.dram_tensor("dbg_x0", [128, N_TILES * N_EMBED], BF16, kind="ExternalOutput")
        dbg["ht"] = nc.dram_tensor("dbg_ht", [128, N_CHUNKS * N_TOK], BF16, kind="ExternalOutput")
        dbg["qt"] = nc.dram_tensor("dbg_qt", [128, N_TOK], BF16, kind="ExternalOutput")
        dbg["kt"] = nc.dram_tensor("dbg_kt", [128, N_TOK], BF16, kind="ExternalOutput")
        dbg["va"] = nc.dram_tensor("dbg_va", [128, N_TILES * V_W], BF16, kind="ExternalOutput")
        dbg["at0"] = nc.dram_tensor("dbg_at0", [128, 2 * 512], BF16, kind="ExternalOutput")
        dbg["po0"] = nc.dram_tensor("dbg_po0", [128, 2 * 512], F32, kind="ExternalOutput")
        dbg["rho0"] = nc.dram_tensor("dbg_rho0", [64, 2 * 512], F32, kind="ExternalOutput")
        dbg["otc"] = nc.dram_tensor("dbg_otc", [128, N_CHUNKS * N_TOK], BF16, kind="ExternalOutput")
        dbg["x1"] = nc.dram_tensor("dbg_x1", [128, N_TILES * N_EMBED], BF16, kind="ExternalOutput")

    AF = mybir.ActivationFunctionType
    OP = mybir.AluOpType

    with tile.TileContext(nc) as tc:
        with tc.tile_pool(name="const", bufs=1) as cst, \
             tc.tile_pool(name="persist", bufs=1) as per, \
             tc.tile_pool(name="work", bufs=3) as wk, \
             tc.tile_pool(name="htile", bufs=9) as hp, \
             tc.tile_pool(name="wts", bufs=4) as wts, \
             tc.tile_pool(name="ps", bufs=4, space="PSUM") as ps:

            # ---- constants ----
            ident = cst.tile([128, 128], BF16)
            nc.sync.dma_start(ident, d["ident"][:, :])
            iota = cst.tile([VOCAB, 1], F32)
            nc.sync.dma_start(iota, d["iota"][:, :])
            negtri = cst.tile([128, 128], BF16)
            nc.sync.dma_start(negtri, d["negtri"][:, :])
            eps_sb = cst.tile([128, 1], F32)
            nc.vector.memset(eps_sb, LN_EPS)
            tok_sb = cst.tile([VOCAB, N_EMBED], BF16)
            nc.sync.dma_start(tok_sb, d["tok_emb"][:, :])

            bias_sb = {}
            for nm, shp in (("vb", [128, N_EMBED]), ("bp", [128, N_EMBED]),
                            ("b2", [128, N_EMBED])):
                if has[nm]:
                    bias_sb[nm] = cst.tile([128, N_LAYERS, shp[1]], F32)
                    nc.sync.dma_start(
                        bias_sb[nm],
                        d[nm].rearrange("l p e -> p l e"))
            if has["b1"]:
                bias_sb["b1"] = cst.tile([128, N_LAYERS, N_MCHUNK], F32)
                nc.sync.dma_start(bias_sb["b1"], d["b1"].rearrange("l p m -> p l m"))
            for nm in ("qb", "kb"):
                if has[nm]:
                    bias_sb[nm] = cst.tile([128, N_LAYERS, 3], F32)
                    nc.sync.dma_start(bias_sb[nm], d[nm].rearrange("l r p -> p l r"))
            if has["blm"]:
                bias_sb["blm"] = cst.tile([128, VOCAB], F32)
                nc.sync.dma_start(bias_sb["blm"], d["blm"][:, :])

            # ---- persistent activations ----
            x = per.tile([128, N_TILES, N_EMBED], BF16)         # residual, token-major
            pos_sb = cst.tile([128, B_LOC, N_EMBED], BF16)
            nc.sync.dma_start(
                pos_sb, d["pos_emb"].rearrange("(a p) e -> p a e", p=128))
            v_aug = per.tile([128, N_TILES, V_W], BF16)         # per-head [ones|V]
            ones_blk = cst.tile([128, 64], BF16)
            nc.vector.memset(ones_blk, 1.0)
            for h in range(N_HEADS):                            # ones stripes
                nc.vector.tensor_copy(
                    v_aug[:, :, h * 128:h * 128 + 64],
                    ones_blk[:, None, :].to_broadcast([128, N_TILES, 64]))

            # ---- embedding: x = onehot(idx) @ tok_emb + pos ----
            for tp in range(N_TILES // 2):
                pe = ps.tile([128, 2, 512], F32, tag="ps")
                for dt in range(2):
                    t = tp * 2 + dt
                    idx_b = wk.tile([VOCAB, 128], F32, tag="idxb")
                    nc.sync.dma_start(
                        idx_b,
                        bass.AP(tensor=d["idxf"], offset=t * 128,
                                ap=[[0, VOCAB], [1, 128]]))
                    oh = wk.tile([VOCAB, 128], BF16, tag="oh")
                    nc.vector.tensor_scalar(out=oh, in0=idx_b, scalar1=iota,
                                            scalar2=None, op0=OP.is_equal)
                    nc.tensor.matmul(pe[:, dt, :N_EMBED], lhsT=oh, rhs=tok_sb,
                                     start=True, stop=False)
                    nc.tensor.matmul(pe[:, dt, :N_EMBED], lhsT=ident,
                                     rhs=pos_sb[:, t % B_LOC, :],
                                     start=False, stop=True)
                    nc.scalar.copy(x[:, t, :], pe[:, dt, :N_EMBED])

            LNG = 4

            def ln_group(dst_hT, src_name, tg):
                """One LN group: stats -> rstd -> apply (bf16) -> 12 PE
                transposes into one 2-bank bf16 PSUM tile (chunks 0,1 in
                bank0, chunk 2 in bank1) -> 2 DVE copies to E-major dst."""
                G = LNG
                mvg = wk.tile([128, G, 2], F32, tag="mv" + src_name)
                for dt in range(G):
                    st = wk.tile([128, 6], F32, tag="bnst")
                    nc.vector.bn_stats(out=st, in_=x[:, tg * G + dt, :])
                    nc.vector.bn_aggr(out=mvg[:, dt, :], in_=st)
                sstd = wk.tile([128, G], F32, tag="sstd")
                nc.scalar.activation(out=sstd, in_=mvg[:, :, 1],
                                     func=AF.Sqrt, bias=eps_sb, scale=1.0)
                rstd = wk.tile([128, G], F32, tag="rstd")
                nc.vector.reciprocal(out=rstd, in_=sstd)
                hts = []
                for dt in range(G):
                    t = tg * G + dt
                    ht = hp.tile([128, N_EMBED], BF16, tag="h")
                    nc.vector.tensor_scalar(
                        out=ht, in0=x[:, t, :],
                        scalar1=mvg[:, dt, 0:1], scalar2=rstd[:, dt:dt + 1],
                        op0=OP.subtract, op1=OP.mult)
                    hts.append(ht)
                pt = ps.tile([128, 2, 1024], BF16, tag="ps")
                for c in range(N_CHUNKS):
                    for dt in range(G):
                        nc.tensor.transpose(
                            pt[:, c // 2, (c % 2) * 512 + dt * 128:
                               (c % 2) * 512 + (dt + 1) * 128],
                            hts[dt][:, c * 128:(c + 1) * 128], ident)
                nc.vector.tensor_copy(
                    dst_hT[:, 0:2, tg * G * 128:(tg + 1) * G * 128],
                    pt[:, 0, :].rearrange("p (a b) -> p a b", a=2))
                nc.vector.tensor_copy(
                    dst_hT[:, 2, tg * G * 128:(tg + 1) * G * 128],
                    pt[:, 1, 0:512])

            def layernorm_to(dst_hT, src_name):
                for tg in range(N_TILES // LNG):
                    ln_group(dst_hT, src_name, tg)

            for layer in range(N_LAYERS):
                if DEBUG_L0 and layer == 0:
                    nc.sync.dma_start(dbg["x0"][:, :], x.rearrange("p a b -> p (a b)"))
                hT = per.tile([128, N_CHUNKS, N_TOK], BF16, tag="ht1")

                # ---- V weights ----
                wv_c = []
                for c in range(N_CHUNKS):
                    w = wts.tile([128, N_EMBED], BF16, tag="wchk", bufs=9)
                    nc.sync.dma_start(w, d["wv"][layer, c * 128:(c + 1) * 128, :])
                    wv_c.append(w)

                def v_pair(tp):
                    pv = ps.tile([128, 2, 512], F32, tag="ps")
                    for dt in range(2):
                        for c in range(N_CHUNKS):
                            nc.tensor.matmul(pv[:, dt, :N_EMBED],
                                             lhsT=hT[:, c, (tp * 2 + dt) * 128:
                                                     (tp * 2 + dt + 1) * 128],
                                             rhs=wv_c[c],
                                             start=(c == 0), stop=(c == N_CHUNKS - 1))
                    # scatter [128, 2, 6, 64] -> per-head V slots (offset 64)
                    src = pv[:, :, :N_EMBED].rearrange("p a (h j) -> p a h j", h=6)
                    dst = v_aug[:, tp * 2:tp * 2 + 2, :].rearrange(
                        "p a (h j) -> p a h j", h=6)[:, :, :, 64:128]
                    if has["vb"]:
                        nc.vector.tensor_tensor(
                            out=dst, in0=src,
                            in1=bias_sb["vb"][:, layer, :].rearrange(
                                "p (h j) -> p h j", h=6)[:, None, :, :]
                            .to_broadcast([128, 2, 6, 64]),
                            op=OP.add)
                    else:
                        nc.scalar.copy(dst, src)

                otc = per.tile([128, N_CHUNKS, N_TOK], BF16, tag="otc")

                def emit_qk_chunks(pair):
                    # ---- QT/KT for this pair: [128, 2048] bf16, head0 on
                    # partitions 0:64, head1 on 64:128. Returns (qkt, list of
                    # 4 chunk closures) so chunks can interleave with the
                    # previous pair's attention groups. ----
                    qkt, chunks = {}, []
                    for nm, wd, bias_nm in (("q", d["wqp"], "qb"),
                                            ("k", d["wkp"], "kb")):
                        wqk = wts.tile([128, N_CHUNKS, 128], BF16, tag="wqk",
                                       bufs=4, name=f"wqk_{nm}")
                        for c in range(N_CHUNKS):
                            nc.sync.dma_start(
                                wqk[:, c, :],
                                wd[layer, pair, c * 128:(c + 1) * 128, :])
                        dstT = per.tile([128, N_TOK], BF16, tag=nm + "t",
                                        bufs=3, name=f"qk_{nm}")
                        qkt[nm] = dstT

                        def chunk(np_, wqk=wqk, dstT=dstT, bias_nm=bias_nm):
                            pq = ps.tile([128, 2, 512], F32, tag="ps", name="pq")
                            for half in range(2):
                                n = np_ * 2 + half
                                for c in range(N_CHUNKS):
                                    nc.tensor.matmul(
                                        pq[:, half, :],
                                        lhsT=wqk[:, c, :],
                                        rhs=hT[:, c, n * 512:(n + 1) * 512],
                                        start=(c == 0), stop=(c == N_CHUNKS - 1))
                            for half in range(2):
                                n = np_ * 2 + half
                                dst = dstT[:, n * 512:(n + 1) * 512]
                                if has[bias_nm]:
                                    nc.scalar.activation(
                                        out=dst, in_=pq[:, half, :],
                                        func=AF.Identity,
                                        bias=bias_sb[bias_nm][:, layer,
                                                             pair:pair + 1],
                                        scale=1.0)
                                else:
                                    nc.scalar.copy(dst, pq[:, half, :])

                        for np_ in range(N_TOK // 1024):
                            chunks.append(lambda np_=np_, chunk=chunk: chunk(np_))
                    return qkt, chunks

                def att_pair(pair, qkt, s0, s1):
                    # ---- attention for two seqs of one head-pair, ki-level
                    # interleaved so the PE stays dense while ACT runs exps.
                    # 3 steps: ki=0 (512), ki=1 (384), ki={2,3} packed into
                    # one bank (cols 0:256 + 256:384) -> 3 exps per group.
                    # S^T for head j lands in bank j of pa; causal mask is
                    # accumulated as -2000 onto each block's diagonal cols. ----
                    seqs = (s0, s1)
                    kT, qT = qkt["k"], qkt["q"]
                    pos = [ps.tile([128, 2, 512], F32, tag="ps", name="po")
                           for _ in range(2)]
                    for step in range(3):
                        pas, ats = {}, {}
                        for g in range(2):
                            s = seqs[g]
                            pa = ps.tile([128, 2, 512], F32, tag="ps", name="pa")
                            at2 = wk.tile([128, 2, 512], BF16, tag="at_sb",
                                          bufs=4, name="at")
                            pas[g], ats[g] = pa, at2
                            if step < 2:
                                ki = step
                                width = 512 - ki * 128
                                kc = s * 512 + ki * 128
                                for j in range(2):
                                    nc.tensor.matmul(
                                        pa[:, j, :width],
                                        lhsT=kT[j * 64:j * 64 + 64, kc:kc + 128],
                                        rhs=qT[j * 64:j * 64 + 64,
                                               kc:s * 512 + 512],
                                        start=True, stop=False)
                                for j in range(2):
                                    nc.tensor.matmul(
                                        pa[:, j, 0:128], lhsT=ident, rhs=negtri,
                                        start=False, stop=True)
                                nc.scalar.activation(
                                    out=at2[:, :, :width], in_=pa[:, :, :width],
                                    func=AF.Exp, scale=SCALE)
                            else:
                                kc3 = s * 512 + 384
                                kc2 = s * 512 + 256
                                for j in range(2):
                                    nc.tensor.matmul(
                                        pa[:, j, 256:384],
                                        lhsT=kT[j * 64:j * 64 + 64, kc3:kc3 + 128],
                                        rhs=qT[j * 64:j * 64 + 64,
                                               kc3:s * 512 + 512],
                                        start=True, stop=False)
                                for j in range(2):
                                    nc.tensor.matmul(
                                        pa[:, j, 256:384], lhsT=ident,
                                        rhs=negtri, start=False, stop=False)
                                # second start=True re-clears only the
                                # has_written bits; ki3 values persist
                                for j in range(2):
                                    nc.tensor.matmul(
                                        pa[:, j, 0:256],
                                        lhsT=kT[j * 64:j * 64 + 64, kc2:kc2 + 128],
                                        rhs=qT[j * 64:j * 64 + 64,
                                               kc2:s * 512 + 512],
                                        start=True, stop=False,
                                        skip_group_check=True)
                                for j in range(2):
                                    nc.tensor.matmul(
                                        pa[:, j, 0:128], lhsT=ident, rhs=negtri,
                                        start=False, stop=True)
                                nc.scalar.activation(
                                    out=at2[:, :, 0:384], in_=pa[:, :, 0:384],
                                    func=AF.Exp, scale=SCALE)
                        for g in range(2):
                            s = seqs[g]
                            pa, at2 = pas[g], ats[g]
                            for j in range(2):
                                h = 2 * pair + j
                                if step < 2:
                                    ki = step
                                    nc.tensor.matmul(
                                        pos[g][:, j, ki * 128:512],
                                        lhsT=v_aug[:, s * 4 + ki,
                                                   h * 128:h * 128 + 128],
                                        rhs=at2[:, j, :512 - ki * 128],
                                        start=(ki == 0), stop=False)
                                else:
                                    nc.tensor.matmul(
                                        pos[g][:, j, 256:512],
                                        lhsT=v_aug[:, s * 4 + 2,
                                                   h * 128:h * 128 + 128],
                                        rhs=at2[:, j, 0:256],
                                        start=False, stop=False)
                                    nc.tensor.matmul(
                                        pos[g][:, j, 384:512],
                                        lhsT=v_aug[:, s * 4 + 3,
                                                   h * 128:h * 128 + 128],
                                        rhs=at2[:, j, 256:384],
                                        start=False, stop=True)
                    for g in range(2):
                        s = seqs[g]
                        po = pos[g]
                        # rows 0:64 of each bank replicate the denominator
                        rho = wk.tile([64, 2, 512], F32, tag="rho", bufs=2,
                                      name="rho")
                        nc.vector.reciprocal_approx_fast(
                            out=rho, in_=po[0:64, :, :])
                        for j in range(2):
                            nc.vector.tensor_tensor(
                                out=otc[64 * j:64 * j + 64, pair,
                                        s * 512:(s + 1) * 512],
                                in0=po[64:128, j, :], in1=rho[:, j, :],
                                op=OP.mult)

                # ---- proj weights (prefetch before attention tail) ----
                wp_c = []
                for c in range(N_CHUNKS):
                    w = wts.tile([128, N_EMBED], BF16, tag="wchk", bufs=9)
                    nc.sync.dma_start(w, d["wp"][layer, c * 128:(c + 1) * 128, :])
                    wp_c.append(w)

                def proj_pair(tp):
                    pp = ps.tile([128, 2, 512], F32, tag="ps")
                    for dt in range(2):
                        t = tp * 2 + dt
                        for c in range(N_CHUNKS):
                            nc.tensor.matmul(
                                pp[:, dt, :N_EMBED],
                                lhsT=otc[:, c, t * 128:(t + 1) * 128],
                                rhs=wp_c[c],
                                start=(c == 0), stop=False)
                        # residual: accumulate x into PSUM via identity matmul
                        nc.tensor.matmul(pp[:, dt, :N_EMBED], lhsT=ident,
                                         rhs=x[:, t, :], start=False, stop=True)
                    if has["bp"]:
                        nc.vector.tensor_tensor(
                            out=x[:, tp * 2:tp * 2 + 2, :],
                            in0=pp[:, :, :N_EMBED],
                            in1=bias_sb["bp"][:, None, layer, :]
                            .to_broadcast([128, 2, N_EMBED]), op=OP.add)
                    else:
                        nc.scalar.copy(x[:, tp * 2:tp * 2 + 2, :],
                                       pp[:, :, :N_EMBED])

                # LN1 groups interleave with V pairs and QK0 chunks; QK of
                # pair p+1 interleaves with attention of pair p (dense PE
                # work hides the exp latency); proj of seq s interleaves
                # with the last pair's attention.
                qkt0, ch0 = emit_qk_chunks(0)
                for tg in range(N_TILES // LNG):
                    ln_group(hT, "ln1", tg)
                    v_pair(2 * tg)
                    v_pair(2 * tg + 1)
                    if tg == 1:
                        ch0[0]()
                        ch0[2]()
                    elif tg == 3:
                        ch0[1]()
                        ch0[3]()
                if DEBUG_L0 and layer == 0:
                    nc.sync.dma_start(dbg["ht"][:, :], hT.rearrange("p a b -> p (a b)"))
                    nc.sync.dma_start(dbg["va"][:, :], v_aug.rearrange("p a b -> p (a b)"))
                # QK of pair p+1 interleaves with attention of pair p (dense
                # PE work hides the exp latency); proj of seq s interleaves
                # with the last pair's attention.
                qkt1, ch1 = emit_qk_chunks(1)
                att_pair(0, qkt0, 0, 1)
                ch1[0]()
                ch1[1]()
                att_pair(0, qkt0, 2, 3)
                ch1[2]()
                ch1[3]()
                qkt2, ch2 = emit_qk_chunks(2)
                att_pair(1, qkt1, 0, 1)
                ch2[0]()
                ch2[1]()
                att_pair(1, qkt1, 2, 3)
                ch2[2]()
                ch2[3]()
                att_pair(2, qkt2, 0, 1)
                for tp in range(4):
                    proj_pair(tp)
                att_pair(2, qkt2, 2, 3)
                for tp in range(4, 8):
                    proj_pair(tp)

                if DEBUG_L0 and layer == 0:
                    nc.sync.dma_start(dbg["otc"][:, :], otc.rearrange("p a b -> p (a b)"))
                    for nm_, t_ in (("qt", qkt2["q"]), ("kt", qkt2["k"])):
                        nc.sync.dma_start(dbg[nm_][:, :], t_[:, :])
                    nc.sync.dma_start(dbg["x1"][:, :], x.rearrange("p a b -> p (a b)"))

                # ---- MLP (LN2 group n feeds MLP1/MLP2 of n-block n) ----
                h2T = per.tile([128, N_CHUNKS, N_TOK], BF16, tag="ht2")
                w1all = wts.tile([128, N_CHUNKS, N_MLP], BF16, tag="w1all", bufs=2)
                for c in range(N_CHUNKS):
                    nc.sync.dma_start(
                        w1all[:, c, :], d["w1"][layer, c * 128:(c + 1) * 128, :])
                w2all = wts.tile([128, N_MCHUNK, N_EMBED], BF16, tag="w2all", bufs=2)
                for m in range(N_MCHUNK):
                    nc.sync.dma_start(
                        w2all[:, m, :], d["w2"][layer, m * 128:(m + 1) * 128, :])
                mlpT = per.tile([128, N_MCHUNK, 512], BF16, tag="mlpt")

                def mlp_block(n):
                    ln_group(h2T, "ln2", n)
                    for mp in range(N_MCHUNK // 2):
                        pm = ps.tile([128, 2, 512], F32, tag="ps")
                        for dm in range(2):
                            m = mp * 2 + dm
                            for c in range(N_CHUNKS):
                                nc.tensor.matmul(
                                    pm[:, dm, :],
                                    lhsT=w1all[:, c, m * 128:(m + 1) * 128],
                                    rhs=h2T[:, c, n * 512:(n + 1) * 512],
                                    start=(c == 0), stop=(c == N_CHUNKS - 1))
                        if has["b1"]:
                            for dm in range(2):
                                nc.scalar.activation(
                                    out=mlpT[:, mp * 2 + dm, :],
                                    in_=pm[:, dm, :], func=AF.Relu,
                                    bias=bias_sb["b1"][:, layer,
                                                       mp * 2 + dm:mp * 2 + dm + 1],
                                    scale=1.0)
                        else:
                            nc.scalar.activation(
                                out=mlpT[:, mp * 2:mp * 2 + 2, :], in_=pm,
                                func=AF.Relu, scale=1.0)
                    for dp in range(2):
                        pw = ps.tile([128, 2, 512], F32, tag="ps")
                        for dt in range(2):
                            t = n * 4 + dp * 2 + dt
                            for m in range(N_MCHUNK):
                                nc.tensor.matmul(
                                    pw[:, dt, :N_EMBED],
                                    lhsT=mlpT[:, m, (dp * 2 + dt) * 128:
                                              (dp * 2 + dt + 1) * 128],
                                    rhs=w2all[:, m, :],
                                    start=(m == 0), stop=False)
                            nc.tensor.matmul(pw[:, dt, :N_EMBED], lhsT=ident,
                                             rhs=x[:, t, :],
                                             start=False, stop=True)
                        t0 = n * 4 + dp * 2
                        if has["b2"]:
                            nc.vector.tensor_tensor(
                                out=x[:, t0:t0 + 2, :],
                                in0=pw[:, :, :N_EMBED],
                                in1=bias_sb["b2"][:, None, layer, :]
                                .to_broadcast([128, 2, N_EMBED]), op=OP.add)
                        else:
                            nc.scalar.copy(x[:, t0:t0 + 2, :],
                                           pw[:, :, :N_EMBED])

                for n in range(N_TOK // 512):
                    mlp_block(n)

            # ---- final LN + LM head ----
            xfT = per.tile([128, N_CHUNKS, N_TOK], BF16, tag="ht1")
            layernorm_to(xfT, "lnf")
            wlm_c = []
            for c in range(N_CHUNKS):
                w = wts.tile([128, VOCAB], BF16, tag="wlm", bufs=3)
                nc.sync.dma_start(w, d["wlm"][c * 128:(c + 1) * 128, :])
                wlm_c.append(w)
            for tp in range(N_TILES // 2):
                pl = ps.tile([128, 2, 512], F32, tag="ps")
                for dt in range(2):
                    for c in range(N_CHUNKS):
                        nc.tensor.matmul(
                            pl[:, dt, :VOCAB],
                            lhsT=xfT[:, c, (tp * 2 + dt) * 128:
                                     (tp * 2 + dt + 1) * 128],
                            rhs=wlm_c[c],
                            start=(c == 0), stop=(c == N_CHUNKS - 1))
                lg = wk.tile([128, 2, VOCAB], F32, tag="lg")
                if has["blm"]:
                    nc.vector.tensor_tensor(
                        out=lg, in0=pl[:, :, :VOCAB],
                        in1=bias_sb["blm"][:, None, :].to_broadcast(
                            [128, 2, VOCAB]), op=OP.add)
                else:
                    nc.vector.tensor_copy(lg, pl[:, :, :VOCAB])
                for dt in range(2):
                    t = tp * 2 + dt
                    nc.sync.dma_start(
                        logits_d[t * 128:(t + 1) * 128, :], lg[:, dt, :])

    nc.compile()
    return nc


_CACHE = {}


def _get_nc(has):
    key = tuple(sorted(has.items()))
    if key not in _CACHE:
        _CACHE[key] = _build(has)
    return _CACHE[key]


def kernel(**inputs):
    shared, has, idx_f = _prep(inputs)
    nc = _get_nc(has)
    in_maps = []
    for core in range(N_CORES):
        m = dict(shared)
        m["idxf"] = idx_f[core]
        in_maps.append(m)
    res = run_bass_kernel_spmd(nc, in_maps, core_ids=list(range(N_CORES)))
    out = np.stack([r["logits"].reshape(B_LOC, T, VOCAB) for r in res.results])
    return out.reshape(B, T, VOCAB)
